# revision 25
# baseline (speedup 1.0000x reference)
"""Fused attention block (QKV proj -> softmax attention -> out proj -> residual+LN)
for B=4, S=2048, D=512, H=8, DH=64 on 8 TRN2 NeuronCores.

Sharding: token-parallel. Core c handles batch b=c//2, query tokens
[(c%2)*1024, (c%2+1)*1024). Each core redundantly computes K/V for its
batch's full 2048-token sequence, runs flash-style attention on-chip, and
writes its disjoint 1024x512 output slice. Zero collectives.

Schedule (vs the 237-272us baseline; measured 219.8us):
- scores are emitted one kc-slot ahead of their exp, so they never queue
  behind the current slot's AV matmuls on the in-order PE (kills the
  ~1.2us exp-stream stall at every head boundary)
- ramp: host pre-chunks xt/weights into [128, 4, .] so each priority load
  is one dma_start; head 0 kc 0-3 scores+exps run in query-half-split form
  so the exp stream starts right after k(0,0)+q(0,0) land (~24us vs ~30us)
- denominator path: raw ctx stashed bf16, den row hopped to partition 0
  via a casting GpSimd DMA, inverted in place with the fast approx DVE
  reciprocal, cast bf16, broadcast over 64 partitions with a K=1 ones-mm
- out-proj in four phases (chunks 0-1 after h3, chunk 2 after h5, head 6's
  half of chunk 3 during h7, head 7's half + LayerNorm in the tail)
- LN final scale-shift as an Identity activation on the post-exp-idle
  Scalar engine (scale=rstd, bias=-mu*rstd)
- all matmuls bf16: fp8 DoubleRow measured NO faster than bf16 on this hw
  (cost tracks the output free size; DR lowers to two passes), and the
  fp8 staging stalled the PE out of its fast pstate
- hw-validated constraints: GPSIMD cannot touch PSUM; DVE tensor ops take
  at most one PSUM input; custom-DVE ops mis-execute cross-partition or at
  nonzero base partitions; only GpSimd DMAs may cast dtype; f32 consumed
  by an f32r matmul is rejected by the BIR verifier
"""

import os
import sys

import numpy as np

for _p in ("/opt/trn_rl_repo",):
    if os.path.isdir(_p) and _p not in sys.path:
        sys.path.insert(0, _p)

import ml_dtypes

import concourse.bacc as bacc
import concourse.bass as bass
import concourse.tile as tile
from concourse import mybir
from concourse.bass_utils import run_bass_kernel_spmd

BF16 = mybir.dt.bfloat16
F32 = mybir.dt.float32
AF = mybir.ActivationFunctionType
ALU = mybir.AluOpType

P = 128        # partitions
D = 512        # hidden dim
DH = 64        # head dim
H = 8          # heads
S = 2048       # tokens per batch element
TQ = 1024      # query tokens per core
B = 4
NCORES = 8
EPS = 1e-5

# Schraudolph exp-on-DVE: bf16 bits = trunc(s*C1 + C2); C2 tuned for the
# truncating f32->int16 convert (max rel err ~4%, zero mean)
EXPC1 = float(0.125 * np.log2(np.e) * 128.0)
EXPC2 = float(16256 - 6.9)
# (head, kc) tiles whose exp runs on the Vector engine instead of Scalar
OFF_EXP = set()  # measured slower: the DVE queue is load-bearing at these slots

TRACE = False
LAST_RESULTS = None
_NC_CACHE = None


def _build():
    nc = bacc.Bacc()

    # x[b] permuted (local queries first), transposed, pre-chunked:
    # xt[p, c*2048+t] = xperm[t, 128c+p]
    xt = nc.declare_dram_parameter("xt", [P, 4 * S], BF16, isOutput=False)
    # xres[p, i*512+d] = xloc[i*128+p, d] + bo[d]
    xres = nc.declare_dram_parameter("xres", [P, 8 * D], BF16, isOutput=False)
    # w*[p, c*512+m] = W[m, 128c+p]   (i.e. W.T pre-chunked)
    wqt = nc.declare_dram_parameter("wqt", [P, 4 * D], BF16, isOutput=False)
    wkt = nc.declare_dram_parameter("wkt", [P, 4 * D], BF16, isOutput=False)
    wvt = nc.declare_dram_parameter("wvt", [P, 4 * D], BF16, isOutput=False)
    wot = nc.declare_dram_parameter("wot", [P, 4 * D], BF16, isOutput=False)
    bqp = nc.declare_dram_parameter("bq", [P, 4], F32, isOutput=False)
    bkp = nc.declare_dram_parameter("bk", [P, 4], F32, isOutput=False)
    bvp = nc.declare_dram_parameter("bv", [P, D], BF16, isOutput=False)
    # out[p, i*512+d] = LN(y)[i*128+p, d]  (gamma/beta applied on host)
    outp = nc.declare_dram_parameter("out", [P, 8 * D], BF16, isOutput=True)

    def dram3(t):
        return t[:, :].rearrange("p (c n) -> p c n", c=4)

    with tile.TileContext(nc) as tc:
        with (
            tc.tile_pool(name="big", bufs=1) as big,
            tc.tile_pool(name="work", bufs=10) as work,
            tc.tile_pool(name="ps_st", bufs=2, space="PSUM") as ps_st,
            tc.tile_pool(name="ps_ctx", bufs=2, space="PSUM") as ps_ctx,
            tc.tile_pool(name="ps_mm", bufs=2, space="PSUM") as ps_mm,
        ):
            wk_sb = big.tile([P, 4, D], BF16)
            xt_sb = big.tile([P, 4, S], BF16)
            wq_sb = big.tile([P, 4, D], BF16)
            wv_sb = big.tile([P, 4, D], BF16)
            wo_sb = big.tile([P, 4, D], BF16)
            bq_sb = big.tile([P, 4], F32)
            bk_sb = big.tile([P, 4], F32)
            bv_sb = big.tile([P, D], BF16)
            xres_sb = big.tile([P, 8, D], BF16)

            # ---------------- loads (priority order) ----------------
            # critical path on sync: one issue per logical chunk
            nc.sync.dma_start(out=bq_sb[:, :], in_=bqp[:, :])
            nc.sync.dma_start(out=bk_sb[:, :], in_=bkp[:, :])
            nc.sync.dma_start(out=wk_sb[:, :, 0:P], in_=dram3(wkt)[:, :, 0:P])
            nc.sync.dma_start(
                out=xt_sb[:, :, 0:D], in_=dram3(xt)[:, :, 0:D]
            )
            nc.sync.dma_start(out=wq_sb[:, :, 0:P], in_=dram3(wqt)[:, :, 0:P])
            nc.sync.dma_start(
                out=xt_sb[:, :, D:2 * D], in_=dram3(xt)[:, :, D:2 * D]
            )
            nc.sync.dma_start(out=wv_sb[:, :, :], in_=dram3(wvt)[:, :, :])
            # bulk loads also on sync, strictly after the priority batch so
            # they never compete for DMA engines with it
            nc.sync.dma_start(
                out=xt_sb[:, :, 2 * D:3 * D], in_=dram3(xt)[:, :, 2 * D:3 * D]
            )
            nc.sync.dma_start(
                out=xt_sb[:, :, 3 * D:4 * D], in_=dram3(xt)[:, :, 3 * D:4 * D]
            )
            nc.sync.dma_start(out=wk_sb[:, :, P:4 * P], in_=dram3(wkt)[:, :, P:4 * P])
            nc.sync.dma_start(out=wq_sb[:, :, P:4 * P], in_=dram3(wqt)[:, :, P:4 * P])
            # off the critical path: gpsimd software DGE (2D APs only)
            nc.gpsimd.dma_start(out=bv_sb[:, :], in_=bvp[:, :])
            for c in range(4):
                nc.gpsimd.dma_start(
                    out=wo_sb[:, c, :], in_=wot[:, c * D:(c + 1) * D]
                )
            nc.gpsimd.dma_start(out=xres_sb[:, :, :], in_=xres[:, :])

            ones_sb = big.tile([1, DH], BF16)
            nc.vector.memset(ones_sb[:, :], 1.0)
            eps_sb = big.tile([P, 1], F32)
            nc.vector.memset(eps_sb[:, :], EPS)

            # V augmented with a ones column per head: [tok, (h, 64 dims + 1)]
            vaug = big.tile([P, 16, H * 65], BF16)
            nc.vector.memset(
                vaug[:, :, :].rearrange("p c (h e) -> p c h e", e=65)[:, :, :, 64:65],
                1.0,
            )

            qt_all = big.tile([P, 4, TQ], BF16)   # Q^T  [dq, tq]
            kt_all = big.tile([P, 4, S], BF16)    # K^T  [dk, t]
            qt_dup = big.tile([P, 4, TQ], BF16)   # partition-swapped copy of Q^T
            kt_dup = big.tile([P, 4, S], BF16)    # partition-swapped copy of K^T
            ctxT = big.tile([P, 4, TQ], BF16)     # normalized ctx^T [dv, tq]
            rawc = big.tile([65, 16, D], BF16)    # unnormalized ctx (slots 0-13)
            den_rec = big.tile([1, 4, D], F32)    # 1/den ring (partition 0)
            rec_c = big.tile([1, 4, D], BF16)     # 1/den in bf16 for the bcast mm
            y_all = big.tile([P, 8, D], F32)      # proj + residual
            mv_all = big.tile([P, 8, 2], F32)     # (mean, var) per token tile
            rstd_all = big.tile([P, 8], F32)
            negb_all = big.tile([P, 8], F32)

            # ---------------- projection emitters ----------------
            def k_proj(m, t4):
                ps = ps_mm.tile([P, D], F32, tag="mm", name="ps_k")
                for kc in range(4):
                    nc.tensor.matmul(
                        ps[:, :],
                        lhsT=wk_sb[:, kc, m * P:(m + 1) * P],
                        rhs=xt_sb[:, kc, t4 * D:(t4 + 1) * D],
                        start=(kc == 0),
                        stop=(kc == 3),
                    )
                nc.vector.tensor_scalar_add(
                    kt_all[:, m, t4 * D:(t4 + 1) * D], ps[:, :], bk_sb[:, m:m + 1]
                )
                sl = slice(t4 * D, (t4 + 1) * D)
                nc.sync.dma_start(out=kt_dup[64:128, m, sl], in_=kt_all[0:64, m, sl])
                nc.sync.dma_start(out=kt_dup[0:64, m, sl], in_=kt_all[64:128, m, sl])

            def q_proj(m, t2):
                ps = ps_mm.tile([P, D], F32, tag="mm", name="ps_q")
                for kc in range(4):
                    nc.tensor.matmul(
                        ps[:, :],
                        lhsT=wq_sb[:, kc, m * P:(m + 1) * P],
                        rhs=xt_sb[:, kc, t2 * D:(t2 + 1) * D],
                        start=(kc == 0),
                        stop=(kc == 3),
                    )
                nc.vector.tensor_scalar_add(
                    qt_all[:, m, t2 * D:(t2 + 1) * D], ps[:, :], bq_sb[:, m:m + 1]
                )
                sl = slice(t2 * D, (t2 + 1) * D)
                nc.sync.dma_start(out=qt_dup[64:128, m, sl], in_=qt_all[0:64, m, sl])
                nc.sync.dma_start(out=qt_dup[0:64, m, sl], in_=qt_all[64:128, m, sl])

            def v_proj(t16):
                ps = ps_mm.tile([P, D], F32, tag="mm", name="ps_v")
                for kc in range(4):
                    nc.tensor.matmul(
                        ps[:, :],
                        lhsT=xt_sb[:, kc, t16 * P:(t16 + 1) * P],
                        rhs=wv_sb[:, kc, :],
                        start=(kc == 0),
                        stop=(kc == 3),
                    )
                nc.vector.tensor_add(
                    out=vaug[:, t16, :].rearrange("p (h e) -> p h e", e=65)[:, :, 0:64],
                    in0=ps[:, :].rearrange("p (h e) -> p h e", e=64),
                    in1=bv_sb[:, :].rearrange("p (h e) -> p h e", e=64),
                )

            # normalize slot s: bf16 recip row (computed at head end straight
            # from the PSUM ctx tile), broadcast across 64 partitions with a
            # K=1 ones-matmul, then scale the raw ctx into ctxT
            def norm_slot(s):
                h, qt2 = s // 2, s % 2
                po = (h % 2) * 64
                chn = h // 2
                nc.vector.tensor_copy(rec_c[0:1, s % 4, :], den_rec[0:1, s % 4, :])
                rb = ps_mm.tile([P, D], F32, tag="mm", name="ps_rb")
                nc.tensor.matmul(
                    rb[0:DH, :],
                    lhsT=ones_sb[0:1, :],
                    rhs=rec_c[0:1, s % 4, :],
                    start=True,
                    stop=True,
                )
                nc.vector.tensor_mul(
                    out=ctxT[po:po + 64, chn, qt2 * D:(qt2 + 1) * D],
                    in0=rawc[0:DH, s, :],
                    in1=rb[0:DH, :],
                )

            # out-projection phases: A = chunks 0,1 (+xres, DVE add),
            # B = chunk 2, C6 = head 6 half of chunk 3, C7 = head 7 half
            # (tail, + LayerNorm + store).  B/C adds run on GpSimd.
            def phase_a(t8):
                ps = ps_mm.tile([P, D], F32, tag="mm", name="ps_oa")
                for c in range(2):
                    nc.tensor.matmul(
                        ps[:, :],
                        lhsT=ctxT[:, c, t8 * P:(t8 + 1) * P],
                        rhs=wo_sb[:, c, :],
                        start=(c == 0),
                        stop=(c == 1),
                    )
                nc.vector.tensor_add(
                    out=y_all[:, t8, :], in0=ps[:, :], in1=xres_sb[:, t8, :]
                )

            def phase_b(t8):
                ps = ps_mm.tile([P, D], F32, tag="mm", name="ps_ob")
                nc.tensor.matmul(
                    ps[:, :],
                    lhsT=ctxT[:, 2, t8 * P:(t8 + 1) * P],
                    rhs=wo_sb[:, 2, :],
                    start=True,
                    stop=True,
                )
                nc.vector.tensor_add(
                    out=y_all[:, t8, :], in0=ps[:, :], in1=y_all[:, t8, :]
                )

            def phase_c6(t8):
                ps = ps_mm.tile([P, D], F32, tag="mm", name="ps_oc6")
                nc.tensor.matmul(
                    ps[:, :],
                    lhsT=ctxT[0:64, 3, t8 * P:(t8 + 1) * P],
                    rhs=wo_sb[0:64, 3, :],
                    start=True,
                    stop=True,
                )
                nc.vector.tensor_add(
                    out=y_all[:, t8, :], in0=ps[:, :], in1=y_all[:, t8, :]
                )

            def phase_c7(t8):
                ps = ps_mm.tile([P, D], F32, tag="mm", name="ps_oc7")
                nc.tensor.matmul(
                    ps[:, :],
                    lhsT=ctxT[64:128, 3, t8 * P:(t8 + 1) * P],
                    rhs=wo_sb[64:128, 3, :],
                    start=True,
                    stop=True,
                )
                nc.vector.tensor_add(
                    out=y_all[:, t8, :], in0=ps[:, :], in1=y_all[:, t8, :]
                )
                stt = work.tile([P, 6], F32, tag="bnst")
                nc.vector.bn_stats(out=stt[:, :], in_=y_all[:, t8, :])
                nc.vector.bn_aggr(out=mv_all[:, t8, :], in_=stt[:, :])
                std1 = work.tile([P, 1], F32, tag="std1")
                nc.scalar.activation(
                    out=std1[:, :], in_=mv_all[:, t8, 1:2], func=AF.Sqrt,
                    bias=eps_sb[:, :], scale=1.0,
                )
                nc.vector.reciprocal(rstd_all[:, t8:t8 + 1], std1[:, :])
                nc.vector.tensor_scalar(
                    out=negb_all[:, t8:t8 + 1], in0=mv_all[:, t8, 0:1],
                    scalar1=rstd_all[:, t8:t8 + 1], scalar2=-1.0,
                    op0=ALU.mult, op1=ALU.mult,
                )
                fin = work.tile([P, D], BF16, tag="fin")
                nc.scalar.activation(
                    out=fin[:, :], in_=y_all[:, t8, :], func=AF.Identity,
                    bias=negb_all[:, t8:t8 + 1], scale=rstd_all[:, t8:t8 + 1],
                )
                nc.sync.dma_start(out=outp[:, t8 * D:(t8 + 1) * D], in_=fin[:, :])

            # ---------------- interleave schedule ----------------
            # ramp: a 128-token mini-K chain plus q(0,0) is enough for the
            # first score; head 0 kc 0-3 run in query-half-split form
            psm = ps_mm.tile([P, D], F32, tag="mm", name="ps_kmini")
            for kc in range(4):
                nc.tensor.matmul(
                    psm[:, 0:P], lhsT=wk_sb[:, kc, 0:P],
                    rhs=xt_sb[:, kc, 0:P],
                    start=(kc == 0), stop=(kc == 3))
            nc.vector.tensor_scalar_add(
                kt_all[:, 0, 0:P], psm[:, 0:P], bk_sb[:, 0:1])
            q_proj(0, 0)
            ramp_pr = []
            ramp_st = []
            for kc in range(4):
                ramp_st.append(
                    ps_st.tile([P, TQ], F32, tag="st", name=f"ramp_st{kc}"))
                ramp_pr.append(
                    work.tile([P, TQ], BF16, tag="probs", name=f"ramp_pr{kc}"))

            def ramp_half(kc, half):
                st, pr = ramp_st[kc], ramp_pr[kc]
                sl = slice(half * D, (half + 1) * D)
                nc.tensor.matmul(
                    st[:, sl],
                    lhsT=kt_all[0:64, 0, kc * P:(kc + 1) * P],
                    rhs=qt_all[0:64, 0, sl],
                    start=True,
                    stop=True,
                )
                nc.scalar.activation(
                    out=pr[:, sl], in_=st[:, sl], func=AF.Exp, scale=0.125
                )

            ramp_half(0, 0)
            # rest of K(0, t4=0): tokens 128-511, then the full dups
            psr = ps_mm.tile([P, D], F32, tag="mm", name="ps_krest")
            for kc in range(4):
                nc.tensor.matmul(
                    psr[:, 0:3 * P], lhsT=wk_sb[:, kc, 0:P],
                    rhs=xt_sb[:, kc, P:4 * P],
                    start=(kc == 0), stop=(kc == 3))
            nc.vector.tensor_scalar_add(
                kt_all[:, 0, P:4 * P], psr[:, 0:3 * P], bk_sb[:, 0:1])
            nc.sync.dma_start(out=kt_dup[64:128, 0, 0:D], in_=kt_all[0:64, 0, 0:D])
            nc.sync.dma_start(out=kt_dup[0:64, 0, 0:D], in_=kt_all[64:128, 0, 0:D])
            ramp_half(1, 0)
            q_proj(0, 1)
            ramp_half(0, 1)
            ramp_half(1, 1)
            k_proj(0, 1)
            ramp_half(2, 0)
            ramp_half(2, 1)
            ramp_half(3, 0)
            ramp_half(3, 1)

            inter = {h: {} for h in range(H)}

            def put(h, kc, u):
                inter[h].setdefault(kc, []).append(u)

            # head 0: V just-in-time + rest of K0
            put(0, 0, lambda: v_proj(0))
            put(0, 0, lambda: v_proj(1))
            for t in range(1, 15):
                put(0, t, lambda t=t: v_proj(t + 1))
            put(0, 5, lambda: k_proj(0, 2))
            put(0, 9, lambda: k_proj(0, 3))
            # K/Q units spread with just-in-time deadlines
            put(1, 0, lambda: k_proj(1, 0))
            put(1, 5, lambda: k_proj(1, 1))
            put(1, 7, lambda: q_proj(1, 0))
            put(1, 10, lambda: q_proj(1, 1))
            put(1, 12, lambda: k_proj(1, 2))
            put(2, 5, lambda: k_proj(1, 3))
            put(2, 7, lambda: k_proj(2, 0))
            put(2, 10, lambda: k_proj(2, 1))
            put(2, 12, lambda: k_proj(2, 2))
            put(3, 0, lambda: k_proj(2, 3))
            put(3, 5, lambda: q_proj(2, 0))
            put(3, 8, lambda: q_proj(2, 1))
            put(3, 11, lambda: k_proj(3, 0))
            put(3, 13, lambda: k_proj(3, 1))
            put(4, 5, lambda: k_proj(3, 2))
            put(4, 13, lambda: k_proj(3, 3))
            put(5, 5, lambda: q_proj(3, 0))
            put(5, 7, lambda: q_proj(3, 1))
            # out-proj phase A (needs slots 0-7, ready h4 kc4)
            put(4, 7, lambda: phase_a(0))
            put(4, 9, lambda: phase_a(1))
            put(4, 11, lambda: phase_a(2))
            put(4, 14, lambda: phase_a(3))
            put(5, 9, lambda: phase_a(4))
            put(5, 11, lambda: phase_a(5))
            put(5, 13, lambda: phase_a(6))
            put(5, 14, lambda: phase_a(7))
            # phase B in head 6, phase C6 in head 7
            for i, kc in enumerate(range(5, 13)):
                put(6, kc, lambda i=i: phase_b(i))
                put(7, kc, lambda i=i: phase_c6(i))
            # deferred normalization of the previous head's two slots
            for h in range(1, 8):
                put(h, 1, lambda s=2 * (h - 1): norm_slot(s))
                put(h, 3, lambda s=2 * (h - 1) + 1: norm_slot(s))

            # ---------------- attention ----------------
            # scores are emitted one slot ahead of their exp so they never
            # queue behind the current slot's AV matmuls on the in-order PE
            pend_st = {}

            def score_emit(h2, kc2):
                po2 = (h2 % 2) * 64
                chn2 = h2 // 2
                dpo2 = 64 - po2
                st = ps_st.tile([P, TQ], F32, tag="st", name=f"st{h2}_{kc2}")
                nc.tensor.matmul(
                    st[:, 0:D],
                    lhsT=kt_all[po2:po2 + 64, chn2, kc2 * P:(kc2 + 1) * P],
                    rhs=qt_all[po2:po2 + 64, chn2, 0:D],
                    start=True,
                    stop=True,
                )
                nc.tensor.matmul(
                    st[:, D:TQ],
                    lhsT=kt_dup[dpo2:dpo2 + 64, chn2, kc2 * P:(kc2 + 1) * P],
                    rhs=qt_dup[dpo2:dpo2 + 64, chn2, D:TQ],
                    start=True,
                    stop=True,
                )
                pend_st[(h2, kc2)] = st

            for h in range(H):
                po = (h % 2) * 64
                chn = h // 2
                cx0 = ps_ctx.tile([65, D], F32, tag="cx")
                cx1 = ps_ctx.tile([65, D], F32, tag="cx")
                for kc in range(16):
                    if kc == 15:
                        if h < 7:
                            score_emit(h + 1, 0)
                    elif not (h == 0 and kc < 3):
                        score_emit(h, kc + 1)
                    for u in inter[h].get(kc, []):
                        u()
                    if h == 0 and kc < 4:
                        pr = ramp_pr[kc]
                    else:
                        st = pend_st.pop((h, kc))
                        pr = work.tile([P, TQ], BF16, tag="probs")
                        if (h, kc) in OFF_EXP:
                            nc.vector.tensor_scalar(
                                out=pr[:, :].bitcast(mybir.dt.int16),
                                in0=st[:, :], scalar1=EXPC1, scalar2=EXPC2,
                                op0=ALU.mult, op1=ALU.add,
                            )
                        else:
                            nc.scalar.activation(
                                out=pr[:, :], in_=st[:, :], func=AF.Exp,
                                scale=0.125
                            )
                    vh = vaug[:, kc, h * 65:(h + 1) * 65]
                    nc.tensor.matmul(
                        cx0[:, :], lhsT=vh, rhs=pr[:, 0:D],
                        start=(kc == 0), stop=(kc == 15),
                    )
                    nc.tensor.matmul(
                        cx1[:, :], lhsT=vh, rhs=pr[:, D:TQ],
                        start=(kc == 0), stop=(kc == 15),
                    )
                # head end: invert dens straight from PSUM, stash raw ctx
                # (head 7 skips the stash; its cx tiles feed the tail muls)
                for qt2, cx in ((0, cx0), (1, cx1)):
                    s = 2 * h + qt2
                    nc.vector.tensor_copy(rawc[0:65, s, :], cx[0:65, :])
                    nc.gpsimd.dma_start(
                        out=den_rec[0:1, s % 4, :], in_=rawc[64:65, s, :]
                    )
                    nc.vector.reciprocal_approx_fast(
                        den_rec[0:1, s % 4, :], den_rec[0:1, s % 4, :]
                    )

            # ---------------- tail ----------------
            norm_slot(14)
            norm_slot(15)
            for t8 in range(8):
                phase_c7(t8)

    nc.compile()
    return nc


def _get_nc():
    global _NC_CACHE
    if _NC_CACHE is None:
        _NC_CACHE = _build()
    return _NC_CACHE


def _chunk4(a):
    # [R, C] -> [R/4-interleaved layout]: out[p, c*C + n] = a[c*128+p, n]
    R, C = a.shape
    return np.ascontiguousarray(
        a.reshape(4, P, C).transpose(1, 0, 2).reshape(P, 4 * C))


def _prep_in_maps(x, Wq, bq, Wk, bk, Wv, bv, Wo, bo):
    bf = ml_dtypes.bfloat16
    x = np.asarray(x, np.float32)
    bo = np.asarray(bo, np.float32)
    wqt_n = _chunk4(np.asarray(Wq, np.float32).T).astype(bf)
    wkt_n = _chunk4(np.asarray(Wk, np.float32).T).astype(bf)
    wvt_n = _chunk4(np.asarray(Wv, np.float32).T).astype(bf)
    wot_n = _chunk4(np.asarray(Wo, np.float32).T).astype(bf)
    bq_n = np.ascontiguousarray(np.asarray(bq, np.float32).reshape(4, P).T)
    bk_n = np.ascontiguousarray(np.asarray(bk, np.float32).reshape(4, P).T)
    bv_n = np.ascontiguousarray(
        np.broadcast_to(np.asarray(bv, np.float32)[None, :], (P, D))).astype(bf)

    in_maps = []
    for c in range(NCORES):
        b = c // 2
        par = c % 2
        xb = x[b]                               # [S, D]
        xloc = xb[par * TQ:(par + 1) * TQ]      # [TQ, D]
        xoth = xb[(1 - par) * TQ:(2 - par) * TQ]
        xperm = np.concatenate([xloc, xoth], axis=0)   # local queries first
        xr = (xloc + bo[None, :]).reshape(8, P, D).transpose(1, 0, 2)
        xres_n = np.ascontiguousarray(xr.reshape(P, 8 * D)).astype(bf)
        in_maps.append({
            "xt": _chunk4(xperm.T).astype(bf),
            "xres": xres_n,
            "wqt": wqt_n, "wkt": wkt_n, "wvt": wvt_n, "wot": wot_n,
            "bq": bq_n, "bk": bk_n, "bv": bv_n,
        })
    return in_maps


def kernel(x, Wq, bq, Wk, bk, Wv, bv, Wo, bo, gamma, beta):
    global LAST_RESULTS
    in_maps = _prep_in_maps(x, Wq, bq, Wk, bk, Wv, bv, Wo, bo)

    nc = _get_nc()
    res = run_bass_kernel_spmd(nc, in_maps, core_ids=list(range(NCORES)), trace=TRACE)
    LAST_RESULTS = res

    outf = np.empty((B, S, D), np.float32)
    for c in range(NCORES):
        b = c // 2
        par = c % 2
        o = np.asarray(res.results[c]["out"], dtype=np.float32)
        o = o.reshape(P, 8, D).transpose(1, 0, 2).reshape(TQ, D)
        outf[b, par * TQ:(par + 1) * TQ, :] = o
    gm = np.asarray(gamma, np.float32)[None, None, :]
    bt = np.asarray(beta, np.float32)[None, None, :]
    return outf * gm + bt


# revision 26
# speedup vs baseline: 1.0130x; 1.0130x over previous
"""Fused attention block (QKV proj -> softmax attention -> out proj -> residual+LN)
for B=4, S=2048, D=512, H=8, DH=64 on 8 TRN2 NeuronCores.

Sharding: token-parallel. Core c handles batch b=c//2, query tokens
[(c%2)*1024, (c%2+1)*1024). Each core redundantly computes K/V for its
batch's full 2048-token sequence, runs flash-style attention on-chip, and
writes its disjoint 1024x512 output slice. Zero collectives.

Schedule (vs the 237-272us baseline; measured 219.8us):
- scores are emitted one kc-slot ahead of their exp, so they never queue
  behind the current slot's AV matmuls on the in-order PE (kills the
  ~1.2us exp-stream stall at every head boundary)
- ramp: host pre-chunks xt/weights into [128, 4, .] so each priority load
  is one dma_start; head 0 kc 0-3 scores+exps run in query-half-split form
  so the exp stream starts right after k(0,0)+q(0,0) land (~24us vs ~30us)
- denominator path: raw ctx stashed bf16, den row hopped to partition 0
  via a casting GpSimd DMA, inverted in place with the fast approx DVE
  reciprocal, cast bf16, broadcast over 64 partitions with a K=1 ones-mm
- out-proj in four phases (chunks 0-1 after h3, chunk 2 after h5, head 6's
  half of chunk 3 during h7, head 7's half + LayerNorm in the tail)
- LN final scale-shift as an Identity activation on the post-exp-idle
  Scalar engine (scale=rstd, bias=-mu*rstd)
- all matmuls bf16: fp8 DoubleRow measured NO faster than bf16 on this hw
  (cost tracks the output free size; DR lowers to two passes), and the
  fp8 staging stalled the PE out of its fast pstate
- hw-validated constraints: GPSIMD cannot touch PSUM; DVE tensor ops take
  at most one PSUM input; custom-DVE ops mis-execute cross-partition or at
  nonzero base partitions; only GpSimd DMAs may cast dtype; f32 consumed
  by an f32r matmul is rejected by the BIR verifier
"""

import os
import sys

import numpy as np

for _p in ("/opt/trn_rl_repo",):
    if os.path.isdir(_p) and _p not in sys.path:
        sys.path.insert(0, _p)

import ml_dtypes

import concourse.bacc as bacc
import concourse.bass as bass
import concourse.tile as tile
from concourse import mybir
from concourse.bass_utils import run_bass_kernel_spmd

BF16 = mybir.dt.bfloat16
F32 = mybir.dt.float32
AF = mybir.ActivationFunctionType
ALU = mybir.AluOpType

P = 128        # partitions
D = 512        # hidden dim
DH = 64        # head dim
H = 8          # heads
S = 2048       # tokens per batch element
TQ = 1024      # query tokens per core
B = 4
NCORES = 8
EPS = 1e-5

# Schraudolph exp-on-DVE: bf16 bits = trunc(s*C1 + C2); C2 tuned for the
# truncating f32->int16 convert (max rel err ~4%, zero mean)
EXPC1 = float(0.125 * np.log2(np.e) * 128.0)
EXPC2 = float(16256 - 6.9)
# (head, kc) tiles whose exp runs on the Vector engine instead of Scalar
OFF_EXP = set()  # measured slower: the DVE queue is load-bearing at these slots

TRACE = False
LAST_RESULTS = None
_NC_CACHE = None


def _build():
    nc = bacc.Bacc()

    # x[b] permuted (local queries first), transposed, pre-chunked:
    # xt[p, c*2048+t] = xperm[t, 128c+p]
    xt = nc.declare_dram_parameter("xt", [P, 4 * S], BF16, isOutput=False)
    # xres[p, i*512+d] = xloc[i*128+p, d] + bo[d]
    xres = nc.declare_dram_parameter("xres", [P, 8 * D], BF16, isOutput=False)
    # w*[p, c*512+m] = W[m, 128c+p]   (i.e. W.T pre-chunked)
    wqt = nc.declare_dram_parameter("wqt", [P, 4 * D], BF16, isOutput=False)
    wkt = nc.declare_dram_parameter("wkt", [P, 4 * D], BF16, isOutput=False)
    wvt = nc.declare_dram_parameter("wvt", [P, 4 * D], BF16, isOutput=False)
    wot = nc.declare_dram_parameter("wot", [P, 4 * D], BF16, isOutput=False)
    bqp = nc.declare_dram_parameter("bq", [P, 4], F32, isOutput=False)
    bkp = nc.declare_dram_parameter("bk", [P, 4], F32, isOutput=False)
    bvp = nc.declare_dram_parameter("bv", [P, D], BF16, isOutput=False)
    # out[p, i*512+d] = LN(y)[i*128+p, d]  (gamma/beta applied on host)
    outp = nc.declare_dram_parameter("out", [P, 8 * D], BF16, isOutput=True)

    def dram3(t):
        return t[:, :].rearrange("p (c n) -> p c n", c=4)

    with tile.TileContext(nc) as tc:
        with (
            tc.tile_pool(name="big", bufs=1) as big,
            tc.tile_pool(name="work", bufs=6) as work,
            tc.tile_pool(name="ps_st", bufs=2, space="PSUM") as ps_st,
            tc.tile_pool(name="ps_ctx", bufs=2, space="PSUM") as ps_ctx,
            tc.tile_pool(name="ps_mm", bufs=2, space="PSUM") as ps_mm,
        ):
            wk_sb = big.tile([P, 4, D], BF16)
            xt_sb = big.tile([P, 4, S], BF16)
            wq_sb = big.tile([P, 4, D], BF16)
            wv_sb = big.tile([P, 4, D], BF16)
            wo_sb = big.tile([P, 4, D], BF16)
            bq_sb = big.tile([P, 4], F32)
            bk_sb = big.tile([P, 4], F32)
            bv_sb = big.tile([P, D], BF16)
            xres_sb = big.tile([P, 8, D], BF16)

            # ---------------- loads (priority order) ----------------
            # critical path on sync: one issue per logical chunk
            nc.sync.dma_start(out=bq_sb[:, :], in_=bqp[:, :])
            nc.sync.dma_start(out=bk_sb[:, :], in_=bkp[:, :])
            nc.sync.dma_start(out=wk_sb[:, :, 0:P], in_=dram3(wkt)[:, :, 0:P])
            nc.sync.dma_start(
                out=xt_sb[:, :, 0:D], in_=dram3(xt)[:, :, 0:D]
            )
            nc.sync.dma_start(out=wq_sb[:, :, 0:P], in_=dram3(wqt)[:, :, 0:P])
            nc.sync.dma_start(
                out=xt_sb[:, :, D:2 * D], in_=dram3(xt)[:, :, D:2 * D]
            )
            nc.sync.dma_start(out=wv_sb[:, :, :], in_=dram3(wvt)[:, :, :])
            # bulk loads also on sync, strictly after the priority batch so
            # they never compete for DMA engines with it
            nc.sync.dma_start(
                out=xt_sb[:, :, 2 * D:3 * D], in_=dram3(xt)[:, :, 2 * D:3 * D]
            )
            nc.sync.dma_start(
                out=xt_sb[:, :, 3 * D:4 * D], in_=dram3(xt)[:, :, 3 * D:4 * D]
            )
            nc.sync.dma_start(out=wk_sb[:, :, P:4 * P], in_=dram3(wkt)[:, :, P:4 * P])
            nc.sync.dma_start(out=wq_sb[:, :, P:4 * P], in_=dram3(wqt)[:, :, P:4 * P])
            # off the critical path: gpsimd software DGE (2D APs only)
            nc.gpsimd.dma_start(out=bv_sb[:, :], in_=bvp[:, :])
            for c in range(4):
                nc.gpsimd.dma_start(
                    out=wo_sb[:, c, :], in_=wot[:, c * D:(c + 1) * D]
                )
            nc.gpsimd.dma_start(out=xres_sb[:, :, :], in_=xres[:, :])

            ones_sb = big.tile([1, DH], BF16)
            nc.vector.memset(ones_sb[:, :], 1.0)
            eps_sb = big.tile([P, 1], F32)
            nc.vector.memset(eps_sb[:, :], EPS)

            # V augmented with a ones column per head: [tok, (h, 64 dims + 1)]
            vaug = big.tile([P, 16, H * 65], BF16)
            nc.vector.memset(
                vaug[:, :, :].rearrange("p c (h e) -> p c h e", e=65)[:, :, :, 64:65],
                1.0,
            )

            qt_all = big.tile([P, 4, TQ], BF16)   # Q^T  [dq, tq]
            kt_all = big.tile([P, 4, S], BF16)    # K^T  [dk, t]
            qt_dup = big.tile([P, 4, TQ], BF16)   # partition-swapped copy of Q^T
            kt_dup = big.tile([P, 4, S], BF16)    # partition-swapped copy of K^T
            ctxT = big.tile([P, 4, TQ], BF16)     # normalized ctx^T [dv, tq]
            rawc = big.tile([65, 16, D], BF16)    # unnormalized ctx (slots 0-13)
            den_rec = big.tile([1, 4, D], F32)    # 1/den ring (partition 0)
            rec_c = big.tile([1, 4, D], BF16)     # 1/den in bf16 for the bcast mm
            y_all = big.tile([P, 8, D], F32)      # proj + residual
            mv_all = big.tile([P, 8, 2], F32)     # (mean, var) per token tile
            rstd_all = big.tile([P, 8], F32)
            negb_all = big.tile([P, 8], F32)

            # ---------------- projection emitters ----------------
            def k_proj(m, t4):
                ps = ps_mm.tile([P, D], F32, tag="mm", name="ps_k")
                for kc in range(4):
                    nc.tensor.matmul(
                        ps[:, :],
                        lhsT=wk_sb[:, kc, m * P:(m + 1) * P],
                        rhs=xt_sb[:, kc, t4 * D:(t4 + 1) * D],
                        start=(kc == 0),
                        stop=(kc == 3),
                    )
                nc.vector.tensor_scalar_add(
                    kt_all[:, m, t4 * D:(t4 + 1) * D], ps[:, :], bk_sb[:, m:m + 1]
                )
                sl = slice(t4 * D, (t4 + 1) * D)
                nc.sync.dma_start(out=kt_dup[64:128, m, sl], in_=kt_all[0:64, m, sl])
                nc.sync.dma_start(out=kt_dup[0:64, m, sl], in_=kt_all[64:128, m, sl])

            def q_proj(m, t2):
                ps = ps_mm.tile([P, D], F32, tag="mm", name="ps_q")
                for kc in range(4):
                    nc.tensor.matmul(
                        ps[:, :],
                        lhsT=wq_sb[:, kc, m * P:(m + 1) * P],
                        rhs=xt_sb[:, kc, t2 * D:(t2 + 1) * D],
                        start=(kc == 0),
                        stop=(kc == 3),
                    )
                nc.vector.tensor_scalar_add(
                    qt_all[:, m, t2 * D:(t2 + 1) * D], ps[:, :], bq_sb[:, m:m + 1]
                )
                sl = slice(t2 * D, (t2 + 1) * D)
                nc.sync.dma_start(out=qt_dup[64:128, m, sl], in_=qt_all[0:64, m, sl])
                nc.sync.dma_start(out=qt_dup[0:64, m, sl], in_=qt_all[64:128, m, sl])

            def v_proj(t16):
                ps = ps_mm.tile([P, D], F32, tag="mm", name="ps_v")
                for kc in range(4):
                    nc.tensor.matmul(
                        ps[:, :],
                        lhsT=xt_sb[:, kc, t16 * P:(t16 + 1) * P],
                        rhs=wv_sb[:, kc, :],
                        start=(kc == 0),
                        stop=(kc == 3),
                    )
                nc.vector.tensor_add(
                    out=vaug[:, t16, :].rearrange("p (h e) -> p h e", e=65)[:, :, 0:64],
                    in0=ps[:, :].rearrange("p (h e) -> p h e", e=64),
                    in1=bv_sb[:, :].rearrange("p (h e) -> p h e", e=64),
                )

            # normalize slot s: bf16 recip row (computed at head end straight
            # from the PSUM ctx tile), broadcast across 64 partitions with a
            # K=1 ones-matmul, then scale the raw ctx into ctxT
            def norm_slot(s):
                h, qt2 = s // 2, s % 2
                po = (h % 2) * 64
                chn = h // 2
                nc.vector.tensor_copy(rec_c[0:1, s % 4, :], den_rec[0:1, s % 4, :])
                rb = ps_mm.tile([P, D], F32, tag="mm", name="ps_rb")
                nc.tensor.matmul(
                    rb[0:DH, :],
                    lhsT=ones_sb[0:1, :],
                    rhs=rec_c[0:1, s % 4, :],
                    start=True,
                    stop=True,
                )
                nc.vector.tensor_mul(
                    out=ctxT[po:po + 64, chn, qt2 * D:(qt2 + 1) * D],
                    in0=rawc[0:DH, s, :],
                    in1=rb[0:DH, :],
                )

            # out-projection phases: A = chunks 0,1 (+xres, DVE add),
            # B = chunk 2, C6 = head 6 half of chunk 3, C7 = head 7 half
            # (tail, + LayerNorm + store).  B/C adds run on GpSimd.
            def phase_a(t8):
                ps = ps_mm.tile([P, D], F32, tag="mm", name="ps_oa")
                for c in range(2):
                    nc.tensor.matmul(
                        ps[:, :],
                        lhsT=ctxT[:, c, t8 * P:(t8 + 1) * P],
                        rhs=wo_sb[:, c, :],
                        start=(c == 0),
                        stop=(c == 1),
                    )
                nc.vector.tensor_add(
                    out=y_all[:, t8, :], in0=ps[:, :], in1=xres_sb[:, t8, :]
                )

            def phase_b(t8):
                ps = ps_mm.tile([P, D], F32, tag="mm", name="ps_ob")
                nc.tensor.matmul(
                    ps[:, :],
                    lhsT=ctxT[:, 2, t8 * P:(t8 + 1) * P],
                    rhs=wo_sb[:, 2, :],
                    start=True,
                    stop=True,
                )
                nc.vector.tensor_add(
                    out=y_all[:, t8, :], in0=ps[:, :], in1=y_all[:, t8, :]
                )

            def phase_c6(t8):
                ps = ps_mm.tile([P, D], F32, tag="mm", name="ps_oc6")
                nc.tensor.matmul(
                    ps[:, :],
                    lhsT=ctxT[0:64, 3, t8 * P:(t8 + 1) * P],
                    rhs=wo_sb[0:64, 3, :],
                    start=True,
                    stop=True,
                )
                nc.vector.tensor_add(
                    out=y_all[:, t8, :], in0=ps[:, :], in1=y_all[:, t8, :]
                )

            def phase_c7(t8):
                ps = ps_mm.tile([P, D], F32, tag="mm", name="ps_oc7")
                nc.tensor.matmul(
                    ps[:, :],
                    lhsT=ctxT[64:128, 3, t8 * P:(t8 + 1) * P],
                    rhs=wo_sb[64:128, 3, :],
                    start=True,
                    stop=True,
                )
                nc.vector.tensor_add(
                    out=y_all[:, t8, :], in0=ps[:, :], in1=y_all[:, t8, :]
                )
                stt = work.tile([P, 6], F32, tag="bnst")
                nc.vector.bn_stats(out=stt[:, :], in_=y_all[:, t8, :])
                nc.vector.bn_aggr(out=mv_all[:, t8, :], in_=stt[:, :])
                std1 = work.tile([P, 1], F32, tag="std1")
                nc.scalar.activation(
                    out=std1[:, :], in_=mv_all[:, t8, 1:2], func=AF.Sqrt,
                    bias=eps_sb[:, :], scale=1.0,
                )
                nc.vector.reciprocal(rstd_all[:, t8:t8 + 1], std1[:, :])
                nc.vector.tensor_scalar(
                    out=negb_all[:, t8:t8 + 1], in0=mv_all[:, t8, 0:1],
                    scalar1=rstd_all[:, t8:t8 + 1], scalar2=-1.0,
                    op0=ALU.mult, op1=ALU.mult,
                )
                fin = work.tile([P, D], BF16, tag="fin")
                nc.scalar.activation(
                    out=fin[:, :], in_=y_all[:, t8, :], func=AF.Identity,
                    bias=negb_all[:, t8:t8 + 1], scale=rstd_all[:, t8:t8 + 1],
                )
                nc.sync.dma_start(out=outp[:, t8 * D:(t8 + 1) * D], in_=fin[:, :])

            # ---------------- interleave schedule ----------------
            # ramp: start the exp stream after only k(0,0)+q(0,0) by
            # processing head 0 kc 0-3 in query-half-split form
            k_proj(0, 0)
            q_proj(0, 0)
            ramp_pr = []
            ramp_st = []
            for kc in range(4):
                ramp_st.append(
                    ps_st.tile([P, TQ], F32, tag="st", name=f"ramp_st{kc}"))
                ramp_pr.append(
                    work.tile([P, TQ], BF16, tag="probs", name=f"ramp_pr{kc}"))

            def ramp_half(kc, half):
                st, pr = ramp_st[kc], ramp_pr[kc]
                sl = slice(half * D, (half + 1) * D)
                nc.tensor.matmul(
                    st[:, sl],
                    lhsT=kt_all[0:64, 0, kc * P:(kc + 1) * P],
                    rhs=qt_all[0:64, 0, sl],
                    start=True,
                    stop=True,
                )
                nc.scalar.activation(
                    out=pr[:, sl], in_=st[:, sl], func=AF.Exp, scale=0.125
                )

            ramp_half(0, 0)
            ramp_half(1, 0)
            q_proj(0, 1)
            ramp_half(0, 1)
            ramp_half(1, 1)
            k_proj(0, 1)
            ramp_half(2, 0)
            ramp_half(2, 1)
            ramp_half(3, 0)
            ramp_half(3, 1)

            inter = {h: {} for h in range(H)}

            def put(h, kc, u):
                inter[h].setdefault(kc, []).append(u)

            # head 0: V just-in-time + rest of K0
            put(0, 0, lambda: v_proj(0))
            put(0, 0, lambda: v_proj(1))
            for t in range(1, 15):
                put(0, t, lambda t=t: v_proj(t + 1))
            put(0, 5, lambda: k_proj(0, 2))
            put(0, 9, lambda: k_proj(0, 3))
            # K/Q units spread with just-in-time deadlines
            put(1, 0, lambda: k_proj(1, 0))
            put(1, 5, lambda: k_proj(1, 1))
            put(1, 7, lambda: q_proj(1, 0))
            put(1, 10, lambda: q_proj(1, 1))
            put(1, 12, lambda: k_proj(1, 2))
            put(2, 5, lambda: k_proj(1, 3))
            put(2, 7, lambda: k_proj(2, 0))
            put(2, 10, lambda: k_proj(2, 1))
            put(2, 12, lambda: k_proj(2, 2))
            put(3, 0, lambda: k_proj(2, 3))
            put(3, 5, lambda: q_proj(2, 0))
            put(3, 8, lambda: q_proj(2, 1))
            put(3, 11, lambda: k_proj(3, 0))
            put(3, 13, lambda: k_proj(3, 1))
            put(4, 5, lambda: k_proj(3, 2))
            put(4, 13, lambda: k_proj(3, 3))
            put(5, 5, lambda: q_proj(3, 0))
            put(5, 7, lambda: q_proj(3, 1))
            # out-proj phase A (needs slots 0-7, ready h4 kc4)
            put(4, 7, lambda: phase_a(0))
            put(4, 9, lambda: phase_a(1))
            put(4, 11, lambda: phase_a(2))
            put(4, 14, lambda: phase_a(3))
            put(5, 9, lambda: phase_a(4))
            put(5, 11, lambda: phase_a(5))
            put(5, 13, lambda: phase_a(6))
            put(5, 14, lambda: phase_a(7))
            # phase B in head 6, phase C6 in head 7
            for i, kc in enumerate(range(5, 13)):
                put(6, kc, lambda i=i: phase_b(i))
                put(7, kc, lambda i=i: phase_c6(i))
            # deferred normalization of the previous head's two slots
            for h in range(1, 8):
                put(h, 1, lambda s=2 * (h - 1): norm_slot(s))
                put(h, 3, lambda s=2 * (h - 1) + 1: norm_slot(s))

            # ---------------- attention ----------------
            # scores are emitted one slot ahead of their exp so they never
            # queue behind the current slot's AV matmuls on the in-order PE
            pend_st = {}

            def score_emit(h2, kc2):
                po2 = (h2 % 2) * 64
                chn2 = h2 // 2
                dpo2 = 64 - po2
                st = ps_st.tile([P, TQ], F32, tag="st", name=f"st{h2}_{kc2}")
                nc.tensor.matmul(
                    st[:, 0:D],
                    lhsT=kt_all[po2:po2 + 64, chn2, kc2 * P:(kc2 + 1) * P],
                    rhs=qt_all[po2:po2 + 64, chn2, 0:D],
                    start=True,
                    stop=True,
                )
                nc.tensor.matmul(
                    st[:, D:TQ],
                    lhsT=kt_dup[dpo2:dpo2 + 64, chn2, kc2 * P:(kc2 + 1) * P],
                    rhs=qt_dup[dpo2:dpo2 + 64, chn2, D:TQ],
                    start=True,
                    stop=True,
                )
                pend_st[(h2, kc2)] = st

            for h in range(H):
                po = (h % 2) * 64
                chn = h // 2
                cx0 = ps_ctx.tile([65, D], F32, tag="cx")
                cx1 = ps_ctx.tile([65, D], F32, tag="cx")
                for kc in range(16):
                    if kc == 15:
                        if h < 7:
                            score_emit(h + 1, 0)
                    elif not (h == 0 and kc < 3):
                        score_emit(h, kc + 1)
                    for u in inter[h].get(kc, []):
                        u()
                    if h == 0 and kc < 4:
                        pr = ramp_pr[kc]
                    else:
                        st = pend_st.pop((h, kc))
                        pr = work.tile([P, TQ], BF16, tag="probs")
                        if (h, kc) in OFF_EXP:
                            nc.vector.tensor_scalar(
                                out=pr[:, :].bitcast(mybir.dt.int16),
                                in0=st[:, :], scalar1=EXPC1, scalar2=EXPC2,
                                op0=ALU.mult, op1=ALU.add,
                            )
                        else:
                            nc.scalar.activation(
                                out=pr[:, :], in_=st[:, :], func=AF.Exp,
                                scale=0.125
                            )
                    vh = vaug[:, kc, h * 65:(h + 1) * 65]
                    nc.tensor.matmul(
                        cx0[:, :], lhsT=vh, rhs=pr[:, 0:D],
                        start=(kc == 0), stop=(kc == 15),
                    )
                    nc.tensor.matmul(
                        cx1[:, :], lhsT=vh, rhs=pr[:, D:TQ],
                        start=(kc == 0), stop=(kc == 15),
                    )
                # head end: invert dens straight from PSUM, stash raw ctx
                # (head 7 skips the stash; its cx tiles feed the tail muls)
                for qt2, cx in ((0, cx0), (1, cx1)):
                    s = 2 * h + qt2
                    nc.vector.tensor_copy(rawc[0:65, s, :], cx[0:65, :])
                    nc.gpsimd.dma_start(
                        out=den_rec[0:1, s % 4, :], in_=rawc[64:65, s, :]
                    )
                    nc.vector.reciprocal_approx_fast(
                        den_rec[0:1, s % 4, :], den_rec[0:1, s % 4, :]
                    )

            # ---------------- tail ----------------
            norm_slot(14)
            norm_slot(15)
            for t8 in range(8):
                phase_c7(t8)

    nc.compile()
    return nc


def _get_nc():
    global _NC_CACHE
    if _NC_CACHE is None:
        _NC_CACHE = _build()
    return _NC_CACHE


def _chunk4(a):
    # [R, C] -> [R/4-interleaved layout]: out[p, c*C + n] = a[c*128+p, n]
    R, C = a.shape
    return np.ascontiguousarray(
        a.reshape(4, P, C).transpose(1, 0, 2).reshape(P, 4 * C))


def _prep_in_maps(x, Wq, bq, Wk, bk, Wv, bv, Wo, bo):
    bf = ml_dtypes.bfloat16
    x = np.asarray(x, np.float32)
    bo = np.asarray(bo, np.float32)
    wqt_n = _chunk4(np.asarray(Wq, np.float32).T).astype(bf)
    wkt_n = _chunk4(np.asarray(Wk, np.float32).T).astype(bf)
    wvt_n = _chunk4(np.asarray(Wv, np.float32).T).astype(bf)
    wot_n = _chunk4(np.asarray(Wo, np.float32).T).astype(bf)
    bq_n = np.ascontiguousarray(np.asarray(bq, np.float32).reshape(4, P).T)
    bk_n = np.ascontiguousarray(np.asarray(bk, np.float32).reshape(4, P).T)
    bv_n = np.ascontiguousarray(
        np.broadcast_to(np.asarray(bv, np.float32)[None, :], (P, D))).astype(bf)

    in_maps = []
    for c in range(NCORES):
        b = c // 2
        par = c % 2
        xb = x[b]                               # [S, D]
        xloc = xb[par * TQ:(par + 1) * TQ]      # [TQ, D]
        xoth = xb[(1 - par) * TQ:(2 - par) * TQ]
        xperm = np.concatenate([xloc, xoth], axis=0)   # local queries first
        xr = (xloc + bo[None, :]).reshape(8, P, D).transpose(1, 0, 2)
        xres_n = np.ascontiguousarray(xr.reshape(P, 8 * D)).astype(bf)
        in_maps.append({
            "xt": _chunk4(xperm.T).astype(bf),
            "xres": xres_n,
            "wqt": wqt_n, "wkt": wkt_n, "wvt": wvt_n, "wot": wot_n,
            "bq": bq_n, "bk": bk_n, "bv": bv_n,
        })
    return in_maps


def kernel(x, Wq, bq, Wk, bk, Wv, bv, Wo, bo, gamma, beta):
    global LAST_RESULTS
    in_maps = _prep_in_maps(x, Wq, bq, Wk, bk, Wv, bv, Wo, bo)

    nc = _get_nc()
    res = run_bass_kernel_spmd(nc, in_maps, core_ids=list(range(NCORES)), trace=TRACE)
    LAST_RESULTS = res

    outf = np.empty((B, S, D), np.float32)
    for c in range(NCORES):
        b = c // 2
        par = c % 2
        o = np.asarray(res.results[c]["out"], dtype=np.float32)
        o = o.reshape(P, 8, D).transpose(1, 0, 2).reshape(TQ, D)
        outf[b, par * TQ:(par + 1) * TQ, :] = o
    gm = np.asarray(gamma, np.float32)[None, None, :]
    bt = np.asarray(beta, np.float32)[None, None, :]
    return outf * gm + bt
